# revision 12
# baseline (speedup 1.0000x reference)
# Trainium2 Bass kernel for nn_MultiHeadAttention_48533130445634 — v3.
#
# Math (faithful to the reference, including its unusual second einsum):
#   scores[b,h,n,m] = softmax_m( (q[b,h,n,:] . k[b,h,m,:]) * 0.125 )
#   out[b,h,m,d]    = (sum_n scores[b,h,n,m]) * v[b,h,m,d]
#
# out = V * colsum(softmax).  Per (b,h), tiled over n (128 rows):
#   S_i = Q_i K^T                 (PE, f32r, PSUM, 1024-wide halves)
#   E_i = exp(S_i*0.125)*2^-5     (fp8e4m3 out; the roofline, split:
#                                  even tiles on ACT (native Exp + rowsum
#                                  via the ACT accumulator), odd tiles on
#                                  the DVE via a custom op p(s)^8 with a
#                                  fitted degree-2 p + in-op accum rowsum)
#   g~_j = GS / rowsum_j          (DVE recip; stored fp8, stride-16 pairs)
#   colsumT[:, t] += E_pair[:,:,128t:..].T @ g~_pair
#                                 (PE fp8 DoubleRow: the whole head's
#                                  colsum accumulates into ONE [128,16]
#                                  PSUM tile at dst partition 0; m-index
#                                  lands on partitions as m = 128 t + p)
#   out[m,d] = colsumT[m] * (v[m,d]/GS)   (GpSimd; V pre-scaled on host)
#
# The exp split halves the scalar-engine wall (~272 -> ~160 us/core); the
# transposed fp8-DoubleRow colsum removes the old 109 us/core colsum matmul
# stream (output free-size 1 per accumulation step) and the cs4 gather.
# The poly approximation's common-mode error cancels in softmax's ratio and
# the colsum averages the rest (end-to-end rel err ~3e-3 incl fp8 E/g).
#
# Sharding: 64 (b,h) pairs across 8 cores, 8 each (SPMD, no cross-core
# comm).  Q/K host-transposed so Dh lands on partitions; V/out use the
# m = 128 t + p layout matching colsumT.

import math
import os

import numpy as np

import concourse.mybir as mybir
import concourse.tile as tile
from concourse import bacc
from concourse.bass_utils import run_bass_kernel_spmd

B, H, N, D = 4, 16, 2048, 64
N_CORES = 8
H_LOC = (B * H) // N_CORES
P = 128
NT = N // P                 # 16 n-tiles; also 16 m-chunks of 128
NP = NT // 2                # 8 tile pairs
SCALE = 0.125
MH = 2
MW = N // MH                # 1024

ESCALE = 2.0 ** -5          # E stored as E*2^-5: fp8e4m3 (max 240) safe
GS = 4096.0                 # g~ = GS/rowsum' in fp8; undone via V/GS on host

# p(u)^8 ~ e^(8u) on |u| <= ~0.8 (u = s/64); coefficients additionally
# fold ESCALE^(1/8) so the op emits e^(s/8)*ESCALE directly.
_C = (1.00847688, 1.06738768, 0.48165367)
_ES = ESCALE ** (1.0 / 8.0)
CF0 = float(_C[0] * _ES)
CF1 = float(_C[1] * _ES / 64.0)
CF2 = float(_C[2] * _ES / (64.0 * 64.0))

TILE_TYPES = "DADA" "DADA" "DADA" "DADA"  # D = DVE custom exp, A = ACT exp

f32 = mybir.dt.float32
f32r = mybir.dt.float32r
f8 = mybir.dt.float8e4
Exp = mybir.ActivationFunctionType.Exp

_EXP_OP = None


def _get_exp_op():
    """Custom DVE op: out = (C0 + x(C1 + x C2))^8, accum_out = row sum."""
    global _EXP_OP
    if _EXP_OP is None:
        from concourse.dve_spec import Spec, Src0, C0, C1, C2, sq, AluOp
        from concourse.dve_spec import lower as dve_lower
        from concourse.dve_spec import _has_src1
        from concourse.dve_ops import DveOp, OPS, get_dve_sub_opcode
        import concourse.dve_ops as dve_ops_mod
        from concourse.dve_uop import DveOpSpec

        poly = C0 + Src0 * (C1 + Src0 * C2)
        spec = Spec(body=sq(sq(sq(poly))), accum=AluOp.ADD)
        op = DveOp("EXP_POLY8_ANT", spec, subdim=False, uops_sha={})
        OPS.append(op)
        dve_ops_mod.CUSTOM_DVE_SPECS[op.name] = spec
        dve_ops_mod._SUB_OPCODE_FOR_NAME[op.name] = (
            dve_ops_mod._CUSTOM_DVE_ROW_BASE + len(OPS) - 1
        )
        for ver in ("v3", "v4"):
            op.uops_sha[ver] = DveOpSpec(
                name=op.name, opcode=get_dve_sub_opcode(op.name),
                uops=dve_lower(spec, ver=ver), rd1_en=_has_src1(spec),
            ).sha(ver)
        _EXP_OP = op
    return _EXP_OP


def _attention_kernel(tc, out, qT, kT, vin):
    nc = tc.nc
    exp_op = _get_exp_op()

    with (
        tc.tile_pool(name="qk", bufs=3) as qk_pool,
        tc.tile_pool(name="ev", bufs=2) as e_pool,
        tc.tile_pool(name="vo", bufs=4) as vo_pool,
        tc.tile_pool(name="st", bufs=2) as st_pool,
        tc.tile_pool(name="s_ps", bufs=3, space="PSUM") as s_pool,
        tc.tile_pool(name="c_ps", bufs=2, space="PSUM") as c_pool,
    ):
        # Exp table preload + PE p-state ramp while the first DMAs land.
        warm = st_pool.tile([P, 1], f32, tag="warm")
        nc.gpsimd.memset(warm[:, :], 0.0)
        nc.scalar.activation(warm[:, :], warm[:, :], func=Exp)
        warm_ps = c_pool.tile([P, NT], f32, tag="csum")
        nc.tensor.matmul(
            warm_ps[0:1, 0:1], lhsT=warm[0:1, 0:1], rhs=warm[0:1, 0:1],
            start=True, stop=True, skip_group_check=True,
        )
        bias_t = st_pool.tile([P, 1], f32, tag="bias")
        nc.gpsimd.memset(bias_t[:, :], float(math.log(ESCALE)))
        # single-row zeros: lhsT/rhs of the c_psT-clearing matmul (start=True
        # zero-marking is bank-row-wide, so clear the whole [P, NT] region
        # with one matmul instead of per-column starts)
        zrow = st_pool.tile([1, P + NT], mybir.dt.bfloat16, tag="zrow")
        nc.vector.memset(zrow[:, :], 0.0)

        loaded = {}

        def emit_loads(h, first=False):
            q_s = qk_pool.tile([D, N], f32r, tag="q")
            k_s = qk_pool.tile([D, N], f32r, tag="k")
            if first:
                parts = [(q_s, qT, 0, P), (k_s, kT, 0, 512),
                         (k_s, kT, 512, MW), (k_s, kT, MW, N),
                         (q_s, qT, P, MW), (q_s, qT, MW, N)]
                for t_s, src, lo, hi in parts:
                    nc.sync.dma_start(out=t_s[:, lo:hi], in_=src[h, :, lo:hi])
            else:
                for half in range(2):
                    sl = slice(half * MW, (half + 1) * MW)
                    nc.sync.dma_start(out=k_s[:, sl], in_=kT[h, :, sl])
                    nc.sync.dma_start(out=q_s[:, sl], in_=qT[h, :, sl])
            # V in the m = 128 t + p layout: v_s[p, t, d] = v[128 t + p, d]
            v_s = vo_pool.tile([P, NT, D], f32, tag="v")
            nc.sync.dma_start(
                out=v_s[:, :, :], in_=vin[h].rearrange("(t p) d -> p t d", p=P)
            )
            e_pairs = [
                e_pool.tile([P, 2, N], f8, tag=f"ep{jp}", name=f"ep{jp}_{h}")
                for jp in range(NP)
            ]
            c_psT = c_pool.tile([P, NT], f32, tag="csum")
            nc.tensor.matmul(
                c_psT[:, :], lhsT=zrow[:, 0:P], rhs=zrow[:, P : P + NT],
                start=True, stop=True, skip_group_check=True,
            )
            loaded[h] = (q_s, k_s, v_s, e_pairs, c_psT)

        emit_loads(0, first=True)

        # pending colsum pair contributions: (min_slot, jp, c_psT, g8, e_pair,
        # tail)
        pending = []
        slot = 0
        entry = [None]  # half-emitted colsum entry, persists across pairs/heads

        def emit_colsum(entry, t_lo, t_hi):
            _, jp, c_psT, g8, e_pair, tail_fn = entry
            for t in range(t_lo, t_hi):
                nc.tensor.matmul(
                    c_psT[:, t : t + 1],
                    lhsT=e_pair[:, :, 128 * t : 128 * (t + 1)],
                    rhs=g8[:, :, jp : jp + 1],
                    start=False,
                    stop=(jp == NP - 1),
                    skip_group_check=True,
                    perf_mode=mybir.MatmulPerfMode.DoubleRow,
                )
            if t_hi == NT and tail_fn is not None:
                tail_fn()

        for h in range(H_LOC):
            last_head = h == H_LOC - 1
            q_s, k_s, v_s, e_pairs, c_psT = loaded.pop(h)
            if not last_head:
                emit_loads(h + 1)

            # rowsum parts in pair layout: [:, i%2, i//2, mh]
            rs_t = st_pool.tile([P, 2, NP, 2], f32, tag="rsp")
            rowsum_t = st_pool.tile([P, 2, NP], f32, tag="rowsum")
            g32_t = st_pool.tile([P, 2, NP], f32, tag="g32")
            g8_t = st_pool.tile([P, 2, NP], f8, tag="g8")

            def make_tail(h=h, c_psT=c_psT, v_s=v_s, last_head=last_head):
                def tail():
                    cs_sb = st_pool.tile([P, NT], f32, tag="cs")
                    nc.vector.tensor_copy(cs_sb[:, :], c_psT[:, :])
                    o_s = vo_pool.tile([P, NT, D], f32, tag="o")
                    eng = nc.vector if last_head else nc.gpsimd
                    out_r = out[h].rearrange("(t p) d -> p t d", p=P)
                    pieces = (
                        tuple((q * (NT // 4), (q + 1) * (NT // 4)) for q in range(4))
                        if last_head else ((0, NT),)
                    )
                    for t0, t1 in pieces:
                        eng.tensor_tensor(
                            o_s[:, t0:t1, :],
                            v_s[:, t0:t1, :],
                            cs_sb[:, t0:t1].unsqueeze(-1).broadcast_to((P, t1 - t0, D)),
                            op=mybir.AluOpType.mult,
                        )
                        nc.sync.dma_start(
                            out=out_r[:, t0:t1, :], in_=o_s[:, t0:t1, :]
                        )

                return tail

            tail_fn = make_tail()

            if last_head:
                batches = [(0, 3), (4, 7), (8, 11), (12, 13), (14, 15)]
            else:
                batches = [(0, NT - 1)]  # one g pass per head; colsum pairs
                # drain through the next head's slots (c_ps/ev double-buffered)
            batch_of = {}
            for b0, b1 in batches:
                for j in range(b0, b1 + 1):
                    batch_of[j] = (b0, b1)

            # Emit per tile-PAIR with interleaved m-halves
            # (A.h0, D.h0, A.h1, D.h1) so every S-ring slot reuse waits on
            # the OTHER engine's exp: the ~500ns fill+sem chain hides behind
            # the opposite engine's work instead of stalling our own.
            for pi in range(NP):
                steps = [(2 * pi, 0), (2 * pi + 1, 0),
                         (2 * pi, 1), (2 * pi + 1, 1)]
                for i, mh in steps:
                    slot += 1
                    ttype = TILE_TYPES[i]
                    e_slot_pair = e_pairs[i // 2]
                    s_ps = s_pool.tile([P, MW], f32, tag="s")
                    for c in range(MW // 512):
                        m0 = mh * MW + c * 512
                        nc.tensor.matmul(
                            s_ps[:, c * 512 : (c + 1) * 512],
                            lhsT=q_s[:, i * P : (i + 1) * P],
                            rhs=k_s[:, m0 : m0 + 512],
                            start=True,
                            stop=True,
                        )
                    if ttype == "A":
                        nc.scalar.activation(
                            e_slot_pair[:, i % 2, mh * MW : (mh + 1) * MW],
                            s_ps[:, :],
                            func=Exp,
                            scale=SCALE,
                            bias=bias_t[:, :],
                            accum_out=rs_t[:, i % 2, i // 2, mh : mh + 1],
                        )
                    else:
                        nc.vector._custom_dve(
                            exp_op,
                            out=e_slot_pair[:, i % 2, mh * MW : (mh + 1) * MW],
                            in0=s_ps[:, :],
                            s0=CF0, s1=CF1, imm2=CF2,
                            accum_out=rs_t[:, i % 2, i // 2, mh : mh + 1],
                        )
                    # drip-feed pending colsum pairs, half a pair per slot
                    if entry[0] is None and pending and pending[0][0] <= slot:
                        entry[0] = pending.pop(0)
                        emit_colsum(entry[0], 0, NT // 2)
                    elif entry[0] is not None:
                        emit_colsum(entry[0], NT // 2, NT)
                        entry[0] = None

                i = 2 * pi + 1
                if i == batch_of[i][1]:  # batch boundary: g for the batch
                    b0, b1 = batch_of[i]
                    jp0, jp1 = b0 // 2, b1 // 2 + 1  # pair range
                    sl = slice(jp0, jp1)
                    nc.vector.tensor_tensor(
                        rowsum_t[:, :, sl],
                        rs_t[:, :, sl, 0],
                        rs_t[:, :, sl, 1],
                        op=mybir.AluOpType.add,
                    )
                    nc.vector.reciprocal(g32_t[:, :, sl], rowsum_t[:, :, sl])
                    nc.vector.tensor_scalar(
                        out=g8_t[:, :, sl], in0=g32_t[:, :, sl],
                        scalar1=GS, scalar2=None,
                        op0=mybir.AluOpType.mult,
                    )
                    lag = 2
                    for idx, jp in enumerate(range(jp0, jp1)):
                        pending.append(
                            (
                                slot + lag + 2 * idx,
                                jp,
                                c_psT,
                                g8_t,
                                e_pairs[jp],
                                tail_fn if jp == NP - 1 else None,
                            )
                        )

            if last_head:
                if entry[0] is not None:
                    emit_colsum(entry[0], NT // 2, NT)
                    entry[0] = None
                while pending:
                    emit_colsum(pending.pop(0), 0, NT)


_NC_CACHE = None


def _get_nc():
    global _NC_CACHE
    if _NC_CACHE is None:
        nc = bacc.Bacc("TRN2", target_bir_lowering=False, debug=False)
        qT = nc.dram_tensor("qT", [H_LOC, D, N], f32r, kind="ExternalInput").ap()
        kT = nc.dram_tensor("kT", [H_LOC, D, N], f32r, kind="ExternalInput").ap()
        vin = nc.dram_tensor("v", [H_LOC, N, D], f32, kind="ExternalInput").ap()
        out = nc.dram_tensor("out", [H_LOC, N, D], f32, kind="ExternalOutput").ap()
        with tile.TileContext(nc) as tc:
            _attention_kernel(tc, out, qT, kT, vin)
        nc.compile()
        # custom-DVE fast-mode flag must be applied to the compiled stream
        fn = nc.m.functions[0]
        for inst in [i for b in fn.blocks for i in b.instructions]:
            if getattr(inst, "op_name", None) == "EXP_POLY8_ANT":
                inst.perf_max = 2
        _NC_CACHE = nc
    return _NC_CACHE


def kernel(q, k, v):
    q = np.asarray(q, dtype=np.float32).reshape(B * H, N, D)
    k = np.asarray(k, dtype=np.float32).reshape(B * H, N, D)
    v = np.asarray(v, dtype=np.float32).reshape(B * H, N, D)
    v_scaled = (v * (1.0 / GS)).astype(np.float32)

    in_maps = []
    for c in range(N_CORES):
        sl = slice(H_LOC * c, H_LOC * (c + 1))
        in_maps.append(
            {
                "qT": np.ascontiguousarray(q[sl].transpose(0, 2, 1)),
                "kT": np.ascontiguousarray(k[sl].transpose(0, 2, 1)),
                "v": np.ascontiguousarray(v_scaled[sl]),
            }
        )

    trace = bool(os.environ.get("KERNEL_TRACE"))
    res = run_bass_kernel_spmd(
        _get_nc(), in_maps, core_ids=list(range(N_CORES)), trace=trace
    )
    if trace:
        print(f"HW exec time: {res.exec_time_ns} ns")
        if res.instructions_and_trace is not None:
            print(f"trace: {res.instructions_and_trace[1]}")

    outs = [r["out"] for r in res.results]
    return np.concatenate(outs, axis=0).reshape(B, H, N, D)


# revision 13
# speedup vs baseline: 1.0030x; 1.0030x over previous
# Trainium2 Bass kernel for nn_MultiHeadAttention_48533130445634 — v3.
#
# Math (faithful to the reference, including its unusual second einsum):
#   scores[b,h,n,m] = softmax_m( (q[b,h,n,:] . k[b,h,m,:]) * 0.125 )
#   out[b,h,m,d]    = (sum_n scores[b,h,n,m]) * v[b,h,m,d]
#
# out = V * colsum(softmax).  Per (b,h), tiled over n (128 rows):
#   S_i = Q_i K^T                 (PE, f32r, PSUM, 1024-wide halves)
#   E_i = exp(S_i*0.125)*2^-5     (fp8e4m3 out; the roofline, split:
#                                  even tiles on ACT (native Exp + rowsum
#                                  via the ACT accumulator), odd tiles on
#                                  the DVE via a custom op p(s)^8 with a
#                                  fitted degree-2 p + in-op accum rowsum)
#   g~_j = GS / rowsum_j          (DVE recip; stored fp8, stride-16 pairs)
#   colsumT[:, t] += E_pair[:,:,128t:..].T @ g~_pair
#                                 (PE fp8 DoubleRow: the whole head's
#                                  colsum accumulates into ONE [128,16]
#                                  PSUM tile at dst partition 0; m-index
#                                  lands on partitions as m = 128 t + p)
#   out[m,d] = colsumT[m] * (v[m,d]/GS)   (GpSimd; V pre-scaled on host)
#
# The exp split halves the scalar-engine wall (~272 -> ~160 us/core); the
# transposed fp8-DoubleRow colsum removes the old 109 us/core colsum matmul
# stream (output free-size 1 per accumulation step) and the cs4 gather.
# The poly approximation's common-mode error cancels in softmax's ratio and
# the colsum averages the rest (end-to-end rel err ~3e-3 incl fp8 E/g).
#
# Sharding: 64 (b,h) pairs across 8 cores, 8 each (SPMD, no cross-core
# comm).  Q/K host-transposed so Dh lands on partitions; V/out use the
# m = 128 t + p layout matching colsumT.

import math
import os

import numpy as np

import concourse.mybir as mybir
import concourse.tile as tile
from concourse import bacc
from concourse.bass_utils import run_bass_kernel_spmd

B, H, N, D = 4, 16, 2048, 64
N_CORES = 8
H_LOC = (B * H) // N_CORES
P = 128
NT = N // P                 # 16 n-tiles; also 16 m-chunks of 128
NP = NT // 2                # 8 tile pairs
SCALE = 0.125
MH = 2
MW = N // MH                # 1024

ESCALE = 2.0 ** -5          # E stored as E*2^-5: fp8e4m3 (max 240) safe
GS = 4096.0                 # g~ = GS/rowsum' in fp8; undone via V/GS on host

# p(u)^8 ~ e^(8u) on |u| <= ~0.8 (u = s/64); coefficients additionally
# fold ESCALE^(1/8) so the op emits e^(s/8)*ESCALE directly.
_C = (1.00847688, 1.06738768, 0.48165367)
_ES = ESCALE ** (1.0 / 8.0)
CF0 = float(_C[0] * _ES)
CF1 = float(_C[1] * _ES / 64.0)
CF2 = float(_C[2] * _ES / (64.0 * 64.0))

TILE_TYPES = "DADA" "DADA" "DADA" "DADA"  # D = DVE custom exp, A = ACT exp

f32 = mybir.dt.float32
f32r = mybir.dt.float32r
f8 = mybir.dt.float8e4
Exp = mybir.ActivationFunctionType.Exp

_EXP_OP = None


def _get_exp_op():
    """Custom DVE op: out = (C0 + x(C1 + x C2))^8, accum_out = row sum."""
    global _EXP_OP
    if _EXP_OP is None:
        from concourse.dve_spec import Spec, Src0, C0, C1, C2, sq, AluOp
        from concourse.dve_spec import lower as dve_lower
        from concourse.dve_spec import _has_src1
        from concourse.dve_ops import DveOp, OPS, get_dve_sub_opcode
        import concourse.dve_ops as dve_ops_mod
        from concourse.dve_uop import DveOpSpec

        poly = C0 + Src0 * (C1 + Src0 * C2)
        spec = Spec(body=sq(sq(sq(poly))), accum=AluOp.ADD)
        op = DveOp("EXP_POLY8_ANT", spec, subdim=False, uops_sha={})
        OPS.append(op)
        dve_ops_mod.CUSTOM_DVE_SPECS[op.name] = spec
        dve_ops_mod._SUB_OPCODE_FOR_NAME[op.name] = (
            dve_ops_mod._CUSTOM_DVE_ROW_BASE + len(OPS) - 1
        )
        for ver in ("v3", "v4"):
            op.uops_sha[ver] = DveOpSpec(
                name=op.name, opcode=get_dve_sub_opcode(op.name),
                uops=dve_lower(spec, ver=ver), rd1_en=_has_src1(spec),
            ).sha(ver)
        _EXP_OP = op
    return _EXP_OP


def _attention_kernel(tc, out, qT, kT, vin):
    nc = tc.nc
    exp_op = _get_exp_op()

    with (
        tc.tile_pool(name="qk", bufs=3) as qk_pool,
        tc.tile_pool(name="ev", bufs=2) as e_pool,
        tc.tile_pool(name="vo", bufs=4) as vo_pool,
        tc.tile_pool(name="st", bufs=2) as st_pool,
        tc.tile_pool(name="s_ps", bufs=3, space="PSUM") as s_pool,
        tc.tile_pool(name="c_ps", bufs=2, space="PSUM") as c_pool,
    ):
        # Exp table preload + PE p-state ramp while the first DMAs land.
        warm = st_pool.tile([P, 1], f32, tag="warm")
        nc.gpsimd.memset(warm[:, :], 0.0)
        nc.scalar.activation(warm[:, :], warm[:, :], func=Exp)
        warm_ps = c_pool.tile([P, NT], f32, tag="csum")
        nc.tensor.matmul(
            warm_ps[0:1, 0:1], lhsT=warm[0:1, 0:1], rhs=warm[0:1, 0:1],
            start=True, stop=True, skip_group_check=True,
        )
        bias_t = st_pool.tile([P, 1], f32, tag="bias")
        nc.gpsimd.memset(bias_t[:, :], float(math.log(ESCALE)))
        # single-row zeros: lhsT/rhs of the c_psT-clearing matmul (start=True
        # zero-marking is bank-row-wide, so clear the whole [P, NT] region
        # with one matmul instead of per-column starts)
        zrow = st_pool.tile([1, P + NT], mybir.dt.bfloat16, tag="zrow")
        nc.vector.memset(zrow[:, :], 0.0)

        loaded = {}

        def emit_loads(h, first=False):
            q_s = qk_pool.tile([D, N], f32r, tag="q")
            k_s = qk_pool.tile([D, N], f32r, tag="k")
            if first:
                parts = [(q_s, qT, 0, P), (k_s, kT, 0, MW),
                         (k_s, kT, MW, N), (q_s, qT, P, MW),
                         (q_s, qT, MW, N)]
                for t_s, src, lo, hi in parts:
                    nc.sync.dma_start(out=t_s[:, lo:hi], in_=src[h, :, lo:hi])
            else:
                for half in range(2):
                    sl = slice(half * MW, (half + 1) * MW)
                    nc.sync.dma_start(out=k_s[:, sl], in_=kT[h, :, sl])
                    nc.sync.dma_start(out=q_s[:, sl], in_=qT[h, :, sl])
            # V in the m = 128 t + p layout: v_s[p, t, d] = v[128 t + p, d]
            v_s = vo_pool.tile([P, NT, D], f32, tag="v")
            nc.sync.dma_start(
                out=v_s[:, :, :], in_=vin[h].rearrange("(t p) d -> p t d", p=P)
            )
            e_pairs = [
                e_pool.tile([P, 2, N], f8, tag=f"ep{jp}", name=f"ep{jp}_{h}")
                for jp in range(NP)
            ]
            c_psT = c_pool.tile([P, NT], f32, tag="csum")
            nc.tensor.matmul(
                c_psT[:, :], lhsT=zrow[:, 0:P], rhs=zrow[:, P : P + NT],
                start=True, stop=True, skip_group_check=True,
            )
            loaded[h] = (q_s, k_s, v_s, e_pairs, c_psT)

        emit_loads(0, first=True)

        # pending colsum pair contributions: (min_slot, jp, c_psT, g8, e_pair,
        # tail)
        pending = []
        slot = 0
        entry = [None]  # half-emitted colsum entry, persists across pairs/heads

        def emit_colsum(entry, t_lo, t_hi):
            _, jp, c_psT, g8, e_pair, tail_fn = entry
            for t in range(t_lo, t_hi):
                nc.tensor.matmul(
                    c_psT[:, t : t + 1],
                    lhsT=e_pair[:, :, 128 * t : 128 * (t + 1)],
                    rhs=g8[:, :, jp : jp + 1],
                    start=False,
                    stop=(jp == NP - 1),
                    skip_group_check=True,
                    perf_mode=mybir.MatmulPerfMode.DoubleRow,
                )
            if t_hi == NT and tail_fn is not None:
                tail_fn()

        for h in range(H_LOC):
            last_head = h == H_LOC - 1
            q_s, k_s, v_s, e_pairs, c_psT = loaded.pop(h)
            if not last_head:
                emit_loads(h + 1)

            # rowsum parts in pair layout: [:, i%2, i//2, mh]
            rs_t = st_pool.tile([P, 2, NP, 2], f32, tag="rsp")
            rowsum_t = st_pool.tile([P, 2, NP], f32, tag="rowsum")
            g32_t = st_pool.tile([P, 2, NP], f32, tag="g32")
            g8_t = st_pool.tile([P, 2, NP], f8, tag="g8")

            def make_tail(h=h, c_psT=c_psT, v_s=v_s, last_head=last_head):
                def tail():
                    cs_sb = st_pool.tile([P, NT], f32, tag="cs")
                    nc.vector.tensor_copy(cs_sb[:, :], c_psT[:, :])
                    o_s = vo_pool.tile([P, NT, D], f32, tag="o")
                    eng = nc.vector if last_head else nc.gpsimd
                    out_r = out[h].rearrange("(t p) d -> p t d", p=P)
                    pieces = (
                        tuple((q * (NT // 4), (q + 1) * (NT // 4)) for q in range(4))
                        if last_head else ((0, NT),)
                    )
                    for t0, t1 in pieces:
                        eng.tensor_tensor(
                            o_s[:, t0:t1, :],
                            v_s[:, t0:t1, :],
                            cs_sb[:, t0:t1].unsqueeze(-1).broadcast_to((P, t1 - t0, D)),
                            op=mybir.AluOpType.mult,
                        )
                        nc.sync.dma_start(
                            out=out_r[:, t0:t1, :], in_=o_s[:, t0:t1, :]
                        )

                return tail

            tail_fn = make_tail()

            if last_head:
                batches = [(0, 3), (4, 7), (8, 11), (12, 13), (14, 15)]
            else:
                batches = [(0, NT - 1)]  # one g pass per head; colsum pairs
                # drain through the next head's slots (c_ps/ev double-buffered)
            batch_of = {}
            for b0, b1 in batches:
                for j in range(b0, b1 + 1):
                    batch_of[j] = (b0, b1)

            # Emit per tile-PAIR with interleaved m-halves
            # (A.h0, D.h0, A.h1, D.h1) so every S-ring slot reuse waits on
            # the OTHER engine's exp: the ~500ns fill+sem chain hides behind
            # the opposite engine's work instead of stalling our own.
            for pi in range(NP):
                steps = [(2 * pi, 0), (2 * pi + 1, 0),
                         (2 * pi, 1), (2 * pi + 1, 1)]
                for i, mh in steps:
                    slot += 1
                    ttype = TILE_TYPES[i]
                    e_slot_pair = e_pairs[i // 2]
                    s_ps = s_pool.tile([P, MW], f32, tag="s")
                    for c in range(MW // 512):
                        m0 = mh * MW + c * 512
                        nc.tensor.matmul(
                            s_ps[:, c * 512 : (c + 1) * 512],
                            lhsT=q_s[:, i * P : (i + 1) * P],
                            rhs=k_s[:, m0 : m0 + 512],
                            start=True,
                            stop=True,
                        )
                    if ttype == "A":
                        nc.scalar.activation(
                            e_slot_pair[:, i % 2, mh * MW : (mh + 1) * MW],
                            s_ps[:, :],
                            func=Exp,
                            scale=SCALE,
                            bias=bias_t[:, :],
                            accum_out=rs_t[:, i % 2, i // 2, mh : mh + 1],
                        )
                    else:
                        nc.vector._custom_dve(
                            exp_op,
                            out=e_slot_pair[:, i % 2, mh * MW : (mh + 1) * MW],
                            in0=s_ps[:, :],
                            s0=CF0, s1=CF1, imm2=CF2,
                            accum_out=rs_t[:, i % 2, i // 2, mh : mh + 1],
                        )
                    # drip-feed pending colsum pairs, half a pair per slot
                    if entry[0] is None and pending and pending[0][0] <= slot:
                        entry[0] = pending.pop(0)
                        emit_colsum(entry[0], 0, NT // 2)
                    elif entry[0] is not None:
                        emit_colsum(entry[0], NT // 2, NT)
                        entry[0] = None

                i = 2 * pi + 1
                if i == batch_of[i][1]:  # batch boundary: g for the batch
                    b0, b1 = batch_of[i]
                    jp0, jp1 = b0 // 2, b1 // 2 + 1  # pair range
                    sl = slice(jp0, jp1)
                    nc.vector.tensor_tensor(
                        rowsum_t[:, :, sl],
                        rs_t[:, :, sl, 0],
                        rs_t[:, :, sl, 1],
                        op=mybir.AluOpType.add,
                    )
                    nc.vector.reciprocal(g32_t[:, :, sl], rowsum_t[:, :, sl])
                    nc.vector.tensor_scalar(
                        out=g8_t[:, :, sl], in0=g32_t[:, :, sl],
                        scalar1=GS, scalar2=None,
                        op0=mybir.AluOpType.mult,
                    )
                    lag = 2
                    for idx, jp in enumerate(range(jp0, jp1)):
                        pending.append(
                            (
                                slot + lag + 2 * idx,
                                jp,
                                c_psT,
                                g8_t,
                                e_pairs[jp],
                                tail_fn if jp == NP - 1 else None,
                            )
                        )

            if last_head:
                if entry[0] is not None:
                    emit_colsum(entry[0], NT // 2, NT)
                    entry[0] = None
                while pending:
                    emit_colsum(pending.pop(0), 0, NT)


_NC_CACHE = None


def _get_nc():
    global _NC_CACHE
    if _NC_CACHE is None:
        nc = bacc.Bacc("TRN2", target_bir_lowering=False, debug=False)
        qT = nc.dram_tensor("qT", [H_LOC, D, N], f32r, kind="ExternalInput").ap()
        kT = nc.dram_tensor("kT", [H_LOC, D, N], f32r, kind="ExternalInput").ap()
        vin = nc.dram_tensor("v", [H_LOC, N, D], f32, kind="ExternalInput").ap()
        out = nc.dram_tensor("out", [H_LOC, N, D], f32, kind="ExternalOutput").ap()
        with tile.TileContext(nc) as tc:
            _attention_kernel(tc, out, qT, kT, vin)
        nc.compile()
        # custom-DVE fast-mode flag must be applied to the compiled stream
        fn = nc.m.functions[0]
        for inst in [i for b in fn.blocks for i in b.instructions]:
            if getattr(inst, "op_name", None) == "EXP_POLY8_ANT":
                inst.perf_max = 2
        _NC_CACHE = nc
    return _NC_CACHE


def kernel(q, k, v):
    q = np.asarray(q, dtype=np.float32).reshape(B * H, N, D)
    k = np.asarray(k, dtype=np.float32).reshape(B * H, N, D)
    v = np.asarray(v, dtype=np.float32).reshape(B * H, N, D)
    v_scaled = (v * (1.0 / GS)).astype(np.float32)

    in_maps = []
    for c in range(N_CORES):
        sl = slice(H_LOC * c, H_LOC * (c + 1))
        in_maps.append(
            {
                "qT": np.ascontiguousarray(q[sl].transpose(0, 2, 1)),
                "kT": np.ascontiguousarray(k[sl].transpose(0, 2, 1)),
                "v": np.ascontiguousarray(v_scaled[sl]),
            }
        )

    trace = bool(os.environ.get("KERNEL_TRACE"))
    res = run_bass_kernel_spmd(
        _get_nc(), in_maps, core_ids=list(range(N_CORES)), trace=trace
    )
    if trace:
        print(f"HW exec time: {res.exec_time_ns} ns")
        if res.instructions_and_trace is not None:
            print(f"trace: {res.instructions_and_trace[1]}")

    outs = [r["out"] for r in res.results]
    return np.concatenate(outs, axis=0).reshape(B, H, N, D)
